# revision 1
# baseline (speedup 1.0000x reference)
"""Trainium2 Bass kernel for nn_Attention_53334903882008 (additive attention).

Reference (per batch b):
  We  = img @ W^T + Wb;  Ue = (hid @ U^T + Ub) broadcast over T
  att = tanh(We + Ue);   e = att @ w + wb
  alpha = softmax_N(e);  phi = sum_n alpha * img      -> [B, T, D]

Sharding: data-parallel over B=8, one batch per NeuronCore; weights
replicated. Per-core dataflow:
  - x = img[b] ([8192, 1024] fp32) is cast fp32->bf16 during the SWDGE DMA
    load, kept in natural [btn, d] layout (rhs of the phi matmul) and
    xbar-DMA-transposed into [d, btn] tiles (moving operand of the We
    matmul, computed as We^T[h, btn]; pre-transposed W stationary).
  - The (Wb+Ub) + U@hid addend is materialized once as a [h, btn%512]
    broadcast tile and added on VectorE before the ScalarE tanh.
  - e = w . att contracts h on partitions (lhsT = w column, M=1 matmuls).
  - Softmax over N=64 runs unnormalized (|e| < ~5, exp cannot overflow):
    exp on ScalarE, grouped per-t sums on VectorE; the 1/sum(t) scale is
    folded into the final phi PSUM->SBUF eviction (per-partition scalar).
  - phi accumulates over all 64 btn-tiles into persistent [t, d] PSUM via
    block-diagonal alpha matrices (built by one VectorE tensor_scalar over
    a constant indicator band; exp(e) reaches partitions via K=1 matmuls).
  - The Tile sem-assigner globally fences Transpose-mode DMAs against
    Copy-mode DMAs (xbar-hang workaround), so casts/transposes are batched
    in chunk groups to amortize the mode-switch drains.
"""

from contextlib import ExitStack

import numpy as np
import ml_dtypes

import concourse.bacc as bacc
import concourse.tile as tile
from concourse import mybir
from concourse.tile import add_dep_helper
from concourse.bass_utils import run_bass_kernel_spmd

B = 8

BF = mybir.dt.bfloat16
F32 = mybir.dt.float32
NPBF = ml_dtypes.bfloat16

T, N, D, H = 128, 64, 1024, 512
BTN = T * N            # 8192
NCH = 8                # chunks over btn
CH = BTN // NCH        # 1024 btn per chunk
JT = CH // 128         # 4 btn-tiles (of 128) per chunk
KT = D // 128          # 8 contraction tiles
HT = H // 128          # 4 h tiles
NI = BTN // 128        # 64 btn-tiles total


def build(nc):
    x_d = nc.dram_tensor("x", [BTN, D], F32, kind="ExternalInput").ap()
    hid_d = nc.dram_tensor("hid", [N, D], F32, kind="ExternalInput").ap()
    wt_d = nc.dram_tensor("wt", [128, KT * HT * 128], BF, kind="ExternalInput").ap()
    ut_d = nc.dram_tensor("ut", [128, KT * HT * 128], BF, kind="ExternalInput").ap()
    wv_d = nc.dram_tensor("wvec", [128, HT], BF, kind="ExternalInput").ap()
    bv_d = nc.dram_tensor("bvec", [1, H], BF, kind="ExternalInput").ap()
    on_d = nc.dram_tensor("ones64", [1, N], BF, kind="ExternalInput").ap()
    i64_d = nc.dram_tensor("i64", [N, N], BF, kind="ExternalInput").ap()
    i128_d = nc.dram_tensor("i128", [128, 128], BF, kind="ExternalInput").ap()
    ind_d = nc.dram_tensor("ind", [N, 512], BF, kind="ExternalInput").ap()
    bw_d = nc.dram_tensor("base", [128, 254], BF, kind="ExternalInput").ap()
    of_d = nc.dram_tensor("onef", [1, 1], F32, kind="ExternalInput").ap()
    phi_d = nc.dram_tensor("phi", [T, D], F32, kind="ExternalOutput").ap()

    with tile.TileContext(nc) as tc, ExitStack() as ctx:
        consts = ctx.enter_context(tc.tile_pool(name="consts", bufs=1))
        xnp = ctx.enter_context(tc.tile_pool(name="xnat", bufs=5))
        xtp = ctx.enter_context(tc.tile_pool(name="xT", bufs=4))
        attp = ctx.enter_context(tc.tile_pool(name="att", bufs=3))
        smal = ctx.enter_context(tc.tile_pool(name="smalls", bufs=4))
        adp = ctx.enter_context(tc.tile_pool(name="adiag", bufs=4))
        psm1 = ctx.enter_context(tc.tile_pool(name="psmm1", bufs=4, space="PSUM"))
        pssm = ctx.enter_context(tc.tile_pool(name="pssml", bufs=2, space="PSUM"))
        psph = ctx.enter_context(tc.tile_pool(name="psphi", bufs=1, space="PSUM"))

        # ---- constants / weights ----
        wt = consts.tile([128, KT, HT, 128], BF)
        nc.sync.dma_start(out=wt, in_=wt_d.rearrange("p (a b c) -> p a b c", a=KT, b=HT))
        ut = consts.tile([128, KT, HT, 128], BF)
        nc.sync.dma_start(out=ut, in_=ut_d.rearrange("p (a b c) -> p a b c", a=KT, b=HT))
        wv = consts.tile([128, HT], BF)
        nc.sync.dma_start(out=wv, in_=wv_d)
        bvec = consts.tile([1, H], BF)
        nc.sync.dma_start(out=bvec, in_=bv_d)
        ones64 = consts.tile([1, N], BF)
        nc.sync.dma_start(out=ones64, in_=on_d)
        i64 = consts.tile([N, N], BF)
        nc.sync.dma_start(out=i64, in_=i64_d)
        i128 = consts.tile([128, 128], BF)
        nc.sync.dma_start(out=i128, in_=i128_d)
        ind = consts.tile([N, 512], BF)
        nc.sync.dma_start(out=ind, in_=ind_d)
        base = consts.tile([128, 254], BF)
        nc.sync.dma_start(out=base, in_=bw_d)
        onef = consts.tile([1, 1], F32)
        nc.sync.dma_start(out=onef, in_=of_d)

        # ---- U_comb = hid @ U^T + (Wb + Ub), kept [64, 512] bf16 ----
        hid_sb = consts.tile([N, D], BF)
        nc.gpsimd.dma_start(out=hid_sb, in_=hid_d)  # cast f32 -> bf16
        # transpose hidden on the PE (hT block = hid_block^T @ I64) instead of
        # an xbar DMA: an early transpose-mode DMA would fence against all the
        # weight/x copy DMAs in flight around it.
        hT = consts.tile([128, KT, N], BF)
        ps_ht = pssm.tile([128, KT * N], F32, tag="sml")
        for kt in range(KT):
            nc.tensor.matmul(
                ps_ht[:, kt * N : (kt + 1) * N],
                lhsT=hid_sb[:, kt * 128 : (kt + 1) * 128],
                rhs=i64,
                start=True,
                stop=True,
            )
        nc.vector.tensor_copy(hT.rearrange("p k n -> p (k n)"), ps_ht)
        ps_u = pssm.tile([N, H], F32, tag="sml")
        for kt in range(KT):
            nc.tensor.matmul(
                ps_u, lhsT=hT[:, kt, :], rhs=ut[:, kt], start=(kt == 0), stop=False
            )
        nc.tensor.matmul(ps_u, lhsT=ones64, rhs=bvec, start=False, stop=True)
        ucomb = consts.tile([N, H], BF)
        nc.scalar.activation(ucomb, ps_u, mybir.ActivationFunctionType.Copy)
        # Materialize U_comb broadcast to the [h, btn] chunk layout once:
        # ucombT_rep[hp, ht, btn] = U_comb[btn%64, ht*128+hp]. Per-chunk the
        # U addend is then a DVE add instead of an extra PE matmul.
        ucombT_rep = consts.tile([128, HT, 512], BF)
        for ht in range(HT):
            ps_rep = pssm.tile([128, 512], F32, tag="sml")
            nc.tensor.matmul(
                ps_rep,
                lhsT=ucomb[:, ht * 128 : (ht + 1) * 128],
                rhs=ind,
                start=True,
                stop=True,
            )
            nc.scalar.activation(
                ucombT_rep[:, ht, :], ps_rep, mybir.ActivationFunctionType.Copy
            )

        # ---- persistent accumulators ----
        s_all = consts.tile([1, T], F32)
        ps_phi0 = psph.tile([T, 512], F32, tag="phi0")
        ps_phi1 = psph.tile([T, 512], F32, tag="phi1")
        ps_phi = [ps_phi0, ps_phi1]

        # ---- main chunk pipeline ----
        # The Tile sem-assigner globally fences Transpose-mode DMAs against
        # Copy-mode DMAs (xbar-hang workaround), so casts and transposes can
        # never overlap. Batch chunks into groups: all casts of a group, then
        # all transposes, then compute - 2 fences per group instead of 2 per
        # chunk, and the DMA pipeline runs a group ahead of the PE.
        # Stage split: A(c) = load/transpose/We-matmuls/tanh/e/exp/sums;
        # B(c) = exp(e)->partitions, alpha-diag, phi matmuls. B(c) is emitted
        # interleaved into the NEXT group's A-compute so the PE never waits
        # on the softmax chain.
        def emit_cast(c):
            xn = xnp.tile([128, JT, D], BF, tag="xn")
            src = (
                x_d.rearrange("(a p) d -> a p d", p=128)[c * JT : (c + 1) * JT]
                .rearrange("a p d -> p a d")
            )
            cast = nc.gpsimd.dma_start(out=xn, in_=src)  # cast f32 -> bf16
            return xn, cast

        def emit_transpose(xn, engine):
            xT = xtp.tile([128, JT, KT, 128], BF, tag="xt")
            tr = engine.dma_start(
                out=xT.rearrange("p j k c -> p (j k) c"),
                in_=xn.rearrange("p j d -> p (j d)"),
                transpose=True,
            )
            return xT, tr

        def emit_pe_transpose(xn):
            # transpose this chunk on the TensorE instead of the xbar DMA -
            # the transpose-mode DMA wall is the kernel's critical path, and
            # the PE has headroom. 32x [128,128] transposes into bf16 PSUM,
            # evicted by ACT copies in [128, 512] groups.
            xT = xtp.tile([128, JT, KT, 128], BF, tag="xt")
            for j in range(JT):
                for kh in range(2):
                    ps_t = psm1.tile([128, 512], BF, tag="mm1")
                    for k2 in range(4):
                        kt = kh * 4 + k2
                        nc.tensor.transpose(
                            ps_t[:, k2 * 128 : (k2 + 1) * 128],
                            xn[:, j, kt * 128 : (kt + 1) * 128],
                            i128,
                        )
                    nc.scalar.activation(
                        xT[:, j, kh * 4 : (kh + 1) * 4, :],
                        ps_t.rearrange("p (a b) -> p a b", a=4),
                        mybir.ActivationFunctionType.Copy,
                    )
            return xT

        def stage_a_compute(c, xn, xT, midwork=None):
            # midwork (the previous chunk's softmax-dependent PE block) is
            # emitted between the two mm1 half-blocks: its sparse tiny-matmul
            # stretch would otherwise trip the HAM MID window at the chunk
            # boundary and re-throttle the PE to 1.2GHz for ~8 matmuls.
            eexp = smal.tile([1, CH], F32, tag="eexp")
            for hf in range(CH // 512):
                if hf == 1 and midwork is not None:
                    midwork()
                ps_e = pssm.tile([1, 512], F32, tag="sml")
                for ht in range(HT):
                    ps = psm1.tile([128, 512], F32, tag="mm1")
                    for kt in range(KT):
                        nc.tensor.matmul(
                            ps,
                            lhsT=wt[:, kt, ht, :],
                            rhs=xT[:, 4 * hf : 4 * hf + 4, kt, :],
                            start=(kt == 0),
                            stop=(kt == KT - 1),
                        )
                    pre = attp.tile([128, 512], F32, tag="pre")
                    nc.vector.tensor_tensor(
                        out=pre, in0=ps, in1=ucombT_rep[:, ht, :], op=mybir.AluOpType.add
                    )
                    att = attp.tile([128, 512], BF, tag="att")
                    nc.scalar.activation(att, pre, mybir.ActivationFunctionType.Tanh)
                    nc.tensor.matmul(
                        ps_e,
                        lhsT=wv[:, ht : ht + 1],
                        rhs=att,
                        start=(ht == 0),
                        stop=(ht == HT - 1),
                    )
                nc.scalar.activation(
                    eexp[0:1, hf * 512 : (hf + 1) * 512],
                    ps_e,
                    mybir.ActivationFunctionType.Exp,
                )
            # per-t sums (groups of 64 along free axis)
            nc.vector.reduce_sum(
                out=s_all[0:1, c * (CH // N) : (c + 1) * (CH // N)],
                in_=eexp.rearrange("p (g n) -> p g n", n=N),
                axis=mybir.AxisListType.X,
            )
            return eexp

        def stage_b(c, xn, eexp):
            # transpose exp(e) slices onto partitions via K=1 matmuls
            ps_a = pssm.tile([128, JT], F32, tag="sml")
            for j in range(JT):
                nc.tensor.matmul(
                    ps_a[:, j : j + 1],
                    lhsT=eexp[0:1, j * 128 : (j + 1) * 128],
                    rhs=onef,
                    start=True,
                    stop=True,
                )
            alpha = smal.tile([128, JT], F32, tag="alpha")
            nc.vector.tensor_copy(alpha, ps_a)

            for j in range(JT):
                ig = c * JT + j  # global btn-tile index
                adiag = adp.tile([128, 128], BF, tag="ad")
                nc.vector.tensor_scalar_mul(
                    adiag,
                    base[:, 126 - 2 * ig : 254 - 2 * ig],
                    alpha[:, j : j + 1],
                )
                for dh in range(2):
                    nc.tensor.matmul(
                        ps_phi[dh],
                        lhsT=adiag,
                        rhs=xn[:, j, dh * 512 : (dh + 1) * 512],
                        start=(ig == 0),
                        stop=(ig == NI - 1),
                    )

        groups = [[0], [1], [2], [3, 4], [5, 6], [7]]
        pe_chunks = {0, 1}  # startup chunks via PE transpose (PE idle then)
        pend = []
        prev_tr = None
        for gi, g in enumerate(groups):
            xns = {}
            for c in g:
                if c == 0:
                    # half-granular first cast: the chunk-0 PE transposes can
                    # start after 2MB instead of 4MB
                    xn = xnp.tile([128, JT, D], BF, tag="xn")
                    src = (
                        x_d.rearrange("(a p) d -> a p d", p=128)[0:JT]
                        .rearrange("a p d -> p a d")
                    )
                    h = JT // 2
                    nc.gpsimd.dma_start(out=xn[:, 0:h, :], in_=src[:, 0:h, :])
                    nc.gpsimd.dma_start(out=xn[:, h:, :], in_=src[:, h:, :])
                else:
                    xn, cast = emit_cast(c)
                xns[c] = xn
            xts = {}
            for c in g:
                if c not in pe_chunks:
                    xT, tr = emit_transpose(xns[c], nc.sync)
                    xts[c] = xT
                    prev_tr = tr
            bq, pend = pend, []
            for idx, c in enumerate(g):
                if c in pe_chunks:
                    xts[c] = emit_pe_transpose(xns[c])
                item = bq[idx] if idx < len(bq) else None
                mw = (lambda it=item: stage_b(*it)) if item is not None else None
                eexp = stage_a_compute(c, xns[c], xts[c], midwork=mw)
                pend.append((c, xns[c], eexp))
            for item in bq[len(g):]:
                stage_b(*item)
        for item in pend:
            stage_b(*item)

        # ---- finalize: phi = ps_phi * (1/s_t) ----
        ps_s = pssm.tile([128, 1], F32, tag="sml")
        nc.tensor.matmul(ps_s, lhsT=s_all, rhs=onef, start=True, stop=True)
        recip = smal.tile([128, 1], F32, tag="recip")
        nc.vector.reciprocal(recip, ps_s)
        phi_sb = consts.tile([T, D], F32)
        for dh in range(2):
            nc.vector.tensor_scalar_mul(
                phi_sb[:, dh * 512 : (dh + 1) * 512], ps_phi[dh], recip
            )
        nc.sync.dma_start(out=phi_d, in_=phi_sb)

    return nc

def prep_consts(W_weight, W_bias, U_weight, U_bias, w_weight):
    def pack_T(M):  # [H, D] -> transposed+tiled [128, KT*HT*128] bf16
        MT = M.T.astype(np.float32)  # [D, H]
        arr = MT.reshape(KT, 128, HT, 128).transpose(1, 0, 2, 3)
        return np.ascontiguousarray(arr.reshape(128, KT * HT * 128)).astype(NPBF)

    base = np.zeros((128, 254), np.float32)
    for p in range(128):
        base[p, 126 + p // 64] = 1.0
    return {
        "wt": pack_T(W_weight),
        "ut": pack_T(U_weight),
        "wvec": np.ascontiguousarray(w_weight[0].reshape(HT, 128).T).astype(NPBF),
        "bvec": (W_bias + U_bias)[None, :].astype(NPBF),
        "ones64": np.ones((1, N), NPBF),
        "i64": np.eye(N, dtype=np.float32).astype(NPBF),
        "i128": np.eye(128, dtype=np.float32).astype(NPBF),
        "ind": np.tile(np.eye(N, dtype=np.float32), (1, 512 // N)).astype(NPBF),
        "base": base.astype(NPBF),
        "onef": np.ones((1, 1), np.float32),
    }


_NC_CACHE = {}


def make_nc(num_devices=B):
    if num_devices not in _NC_CACHE:
        nc = bacc.Bacc(
            "TRN2", target_bir_lowering=False, debug=False, num_devices=num_devices
        )
        build(nc)
        nc.compile()
        _NC_CACHE[num_devices] = nc
    return _NC_CACHE[num_devices]


def prep_in_maps(img_features, hidden_state, consts):
    return [
        {
            "x": np.ascontiguousarray(
                img_features[b].reshape(BTN, D), dtype=np.float32
            ),
            "hid": np.ascontiguousarray(hidden_state[:, b, :], dtype=np.float32),
            **consts,
        }
        for b in range(B)
    ]


def run(inputs, trace=False, tmpdir=None):
    """Run the SPMD kernel; returns (phi [B,T,D] fp32, BassKernelResults)."""
    inputs = {k: np.asarray(v) for k, v in inputs.items()}
    consts = prep_consts(
        inputs["W_weight"], inputs["W_bias"], inputs["U_weight"], inputs["U_bias"],
        inputs["w_weight"],
    )
    in_maps = prep_in_maps(inputs["img_features"], inputs["hidden_state"], consts)
    nc = make_nc(B)
    last_err = None
    for attempt in range(3):
        try:
            res = run_bass_kernel_spmd(
                nc, in_maps, core_ids=list(range(B)), trace=trace, tmpdir=tmpdir
            )
            break
        except Exception as e:  # transient NRT_EXEC_UNIT_UNRECOVERABLE etc.
            last_err = e
            if "UNRECOVERABLE" not in str(e) and "UNAVAILABLE" not in str(e):
                raise
    else:
        raise last_err
    phi = np.stack([res.results[b]["phi"] for b in range(B)]).astype(np.float32)
    return phi, res


def kernel(**inputs) -> np.ndarray:
    phi, _ = run(inputs, trace=False)
    return phi

